# revision 17
# baseline (speedup 1.0000x reference)
"""Causal multi-head self-attention with RoPE on 8 Trainium2 NeuronCores.

Sharding: core = (batch b, head-group g) with b = core//2, g = core%2.
Each core computes QKV projections for its batch element restricted to its
8 heads (512 of 1024 projection rows), RoPE, causal attention, and the
partial output projection y_g = attn_g @ wo[:, g*512:(g+1)*512].T.  The host
sums the two head-group partials per batch element.

v3 redesign (vs the 466us v2 baseline):
v2's trace showed the PE at half clock (HAM K=4/8) for ~270us: phase 2's
per-unit PE work (~0.5us) sat under the ACT exp rate (~0.7us/unit), the
micro-idles kept re-throttling the PE, and cold matmuls (427ns vs 213ns)
then became the critical path.  v3 merges the phases: the projection
pipeline for s-blocks 4..15 is decomposed into small closures that are
pumped between attention score/PV matmuls, so the PE always has dense
independent work while ACT exps drain.  Engine totals land ~balanced
(PE ~215us, ACT ~210us, DVE ~200us).
- scores for the head pair (2hp, 2hp+1) sit in row groups 0-1 / 2-3
  (contraction 64 at base partitions 0/64) and execute concurrently.
- PV accumulates both heads of a pair into one 2-bank PSUM tile with
  per-bank start/stop flags; pv matmuls lag scores by 2 key-blocks.
- PSUM: proj 2 banks, scores 2, {transpose-chain, oproj} shared 2, PV 2.
"""
import math
import os
from contextlib import ExitStack

import numpy as np
import ml_dtypes

import concourse.bass as bass
import concourse.tile as tile
from concourse import bacc, mybir
from concourse import masks
from concourse.bass_utils import run_bass_kernel_spmd

F32 = mybir.dt.float32
BF16 = mybir.dt.bfloat16
EXPF = mybir.ActivationFunctionType.Exp

D = 1024          # d_model
NH = 16           # heads total
DK = 64           # head dim
S = 2048          # sequence
B = 4             # batch
THETA = 10000.0
HPG = 8           # heads per group (2 groups over 8 cores with 4 batches)
W = HPG * DK      # 512: local projection width
NSB = S // 128    # 16 s-blocks
NQG = 4           # 512-wide q groups
ESH = -3.0        # exp shift: exp(s/8 - 3); cancels in normalization
SCL = 1.0 / math.sqrt(DK)

MODE = "v4-pairexp (bf16)"
TRACE = bool(int(os.environ.get("KTRACE", "0")))

_cache = {}


def build_nc():
    nc = bacc.Bacc(None, target_bir_lowering=False, debug=False)

    xt = nc.dram_tensor("xt", [D, S], BF16, kind="ExternalInput")
    wqt = nc.dram_tensor("wqt", [D, W], BF16, kind="ExternalInput")
    wkt = nc.dram_tensor("wkt", [D, W], BF16, kind="ExternalInput")
    wvt = nc.dram_tensor("wvt", [D, W], BF16, kind="ExternalInput")
    wot = nc.dram_tensor("wot", [W, D], BF16, kind="ExternalInput")
    cosb = nc.dram_tensor("cosb", [S, W // 2], F32, kind="ExternalInput")
    sinb = nc.dram_tensor("sinb", [S, W // 2], F32, kind="ExternalInput")
    yp = nc.dram_tensor("yp", [S, D], F32, kind="ExternalOutput")

    xt3 = xt[:].rearrange("(jo p) s -> p jo s", p=128)       # [128, 8, S]
    wqt3 = wqt[:].rearrange("(jo p) i -> p jo i", p=128)     # [128, 8, W]
    wkt3 = wkt[:].rearrange("(jo p) i -> p jo i", p=128)
    wvt3 = wvt[:].rearrange("(jo p) i -> p jo i", p=128)
    wot3 = wot[:].rearrange("(jo p) i -> p jo i", p=128)     # [128, 4, D]

    with tile.TileContext(nc, pool_alloc_mode="queue") as tc, \
            ExitStack() as ctx:
        persist = ctx.enter_context(tc.tile_pool(name="persist", bufs=1))
        bsh = persist.tile([128, 1], F32, name="bsh")
        nc.gpsimd.memset(bsh, ESH)

        # persistent activations: q^T / k^T as [128, slab, S] (slab j holds
        # the 128 dims of heads (2j, 2j+1)); v s-major in s-block PAIRS with
        # a ones column per head for the softmax denominator.
        qT4 = persist.tile([128, 4, S], BF16, name="qT4")
        kT4 = persist.tile([128, 4, S], BF16, name="kT4")
        vtp = [persist.tile([128, 2, HPG, DK + 1], BF16, name=f"vtp{i}")
               for i in range(NSB // 2)]

        wp = ctx.enter_context(tc.tile_pool(name="wp", bufs=1))
        p1t = ctx.enter_context(tc.tile_pool(name="p1t", bufs=3))
        p2t = ctx.enter_context(tc.tile_pool(name="p2t", bufs=3))
        aqp = ctx.enter_context(tc.tile_pool(name="aqp", bufs=2))
        expp = ctx.enter_context(tc.tile_pool(name="exp", bufs=6))
        # PSUM: proj 2 banks; {score-pairs, transpose-chain, oproj-pair}
        # share one 2-slot pool of 2-bank slots; PV pair 2 banks.
        ppp = ctx.enter_context(tc.tile_pool(name="ppp", bufs=2,
                                             space="PSUM"))
        scp = ctx.enter_context(tc.tile_pool(name="scp", bufs=2,
                                             space="PSUM"))
        pvp = ctx.enter_context(tc.tile_pool(name="pvp", bufs=1,
                                             space="PSUM"))

        # ---- weights: wq as 8 per-chunk tiles on the scalar queue so the
        # first projection matmul waits on 128KB, not 1MB; k/v/o on gpsimd
        wq_j = [wp.tile([128, W], BF16, name=f"wq_j{jo}") for jo in range(8)]
        for jo in range(8):
            nc.scalar.dma_start(wq_j[jo][:], wqt3[:, jo, :])
        wk_s = wp.tile([128, 8, W], BF16, name="wk_s")
        wv_s = wp.tile([128, 8, W], BF16, name="wv_s")
        wo_s = wp.tile([128, 4, D], BF16, name="wo_s")
        nc.gpsimd.dma_start(wk_s[:], wkt3[:])
        nc.gpsimd.dma_start(wv_s[:], wvt3[:])
        nc.gpsimd.dma_start(wo_s[:], wot3[:])

        def rope(ps, outt, c3, s3):
            # ps: [128, W] PSUM (pre-RoPE proj, s-major, heads as
            # [evens(32) | odds(32)] blocks); outt: [128, W] SBUF bf16
            pe = ps.rearrange("p (h eo c) -> p h eo c", eo=2, c=32)
            ein, oin = pe[:, :, 0, :], pe[:, :, 1, :]
            oe = outt.rearrange("p (h eo c) -> p h eo c", eo=2, c=32)
            eout, oout = oe[:, :, 0, :], oe[:, :, 1, :]
            ra = p1t.tile([128, 8, 32], F32, name="ra", tag="ra")
            rb = p1t.tile([128, 8, 32], F32, name="rb", tag="rb")
            nc.vector.tensor_mul(ra, ein, c3)
            nc.vector.tensor_mul(rb, oin, s3)
            nc.vector.tensor_sub(eout, ra, rb)
            rc = p1t.tile([128, 8, 32], F32, name="rc", tag="rc")
            rd = p1t.tile([128, 8, 32], F32, name="rd", tag="rd")
            nc.vector.tensor_mul(rc, ein, s3)
            nc.vector.tensor_mul(rd, oin, c3)
            nc.vector.tensor_add(oout, rc, rd)

        def proj_closures(sb, pro=False):
            """Projection pipeline for s-block sb, as a list of small
            closures (each ~0.5-1us of one engine) pumped between
            attention matmuls.  In the prologue (pro=True), pq/pk live in
            halves of an (otherwise idle) scp pair slot so the PE never
            waits for a RoPE drain."""
            s0 = sb * 128
            st = {}

            def get_psum(key):
                if key not in st:
                    if pro and key in ("pq", "pk"):
                        if "qkpair" not in st:
                            st["qkpair"] = scp.tile([128, 2, 512], F32,
                                                    name="qkp", tag="sc")
                        st[key] = st["qkpair"][:, 0 if key == "pq" else 1, :]
                    else:
                        st[key] = ppp.tile([128, W], F32, name=key,
                                           tag="pp")
                return st[key]

            def load():
                xs = p1t.tile([128, 8, 128], BF16, name="xs", tag="xs")
                nc.sync.dma_start(xs[:], xt3[:, :, s0:s0 + 128])
                cs = p1t.tile([128, W // 2], F32, name="cs", tag="cs")
                nc.sync.dma_start(cs[:], cosb[s0:s0 + 128, :])
                sn = p1t.tile([128, W // 2], F32, name="sn", tag="sn")
                nc.sync.dma_start(sn[:], sinb[s0:s0 + 128, :])
                st["xs"], st["cs"], st["sn"] = xs, cs, sn

            def mk_mm(key, wsrc, lo, hi):
                def mm():
                    dst, xs = get_psum(key), st["xs"]
                    for jo in range(lo, hi):
                        if isinstance(wsrc, list):
                            w = wsrc[jo][:]
                        else:
                            w = wsrc[:, jo, :]
                        nc.tensor.matmul(dst[:], xs[:, jo, :], w,
                                         start=(jo == 0), stop=(jo == 7))
                return mm

            def ropeq():
                q_ro = p1t.tile([128, W], BF16, name="q_ro", tag="qro",
                                bufs=2)
                c3 = st["cs"].rearrange("p (h c) -> p h c", c=32)
                s3 = st["sn"].rearrange("p (h c) -> p h c", c=32)
                rope(st["pq"], q_ro, c3, s3)
                st["q_ro"] = q_ro

            def ropek():
                k_ro = p1t.tile([128, W], BF16, name="k_ro", tag="kro",
                                bufs=2)
                c3 = st["cs"].rearrange("p (h c) -> p h c", c=32)
                s3 = st["sn"].rearrange("p (h c) -> p h c", c=32)
                rope(st["pk"], k_ro, c3, s3)
                st["k_ro"] = k_ro

            def vev():
                # v eviction (+ ones column per head) on DVE
                v3 = vtp[sb // 2][:, sb % 2, :, :]       # [128, 8, 65]
                nc.vector.tensor_copy(
                    v3[:, :, 0:DK],
                    st["pv"].rearrange("p (h c) -> p h c", c=DK))
                nc.gpsimd.memset(v3[:, :, DK:DK + 1].bitcast(mybir.dt.uint16),
                                 0x3F80)

            def trans():
                # d-major slabs via DMA xbar transposes (scalar HWDGE
                # queue, kept free of plain copies to avoid xbar-mode
                # thrash) — no PE/DVE time
                q_ro, k_ro = st["q_ro"], st["k_ro"]
                for pr in range(4):
                    nc.scalar.dma_start(
                        qT4[:, pr, s0:s0 + 128],
                        q_ro[:, pr * 128:(pr + 1) * 128], transpose=True)
                    nc.scalar.dma_start(
                        kT4[:, pr, s0:s0 + 128],
                        k_ro[:, pr * 128:(pr + 1) * 128], transpose=True)

            return [load,
                    mk_mm("pq", wq_j, 0, 4), mk_mm("pq", wq_j, 4, 8),
                    ropeq,
                    mk_mm("pk", wk_s, 0, 4), mk_mm("pk", wk_s, 4, 8),
                    ropek,
                    mk_mm("pv", wv_s, 0, 4), mk_mm("pv", wv_s, 4, 8),
                    vev, trans]

        # absorb the ~2.7us exp table load off the critical path
        wdum = p2t.tile([1, 1], F32, name="wdum", tag="wdum")
        nc.scalar.activation(wdum[:], bsh[0:1, 0:1], EXPF)

        # prologue: s-blocks 0-3 run eagerly (loads first so the sync
        # queue streams x while the PE chews); 4-15 queued as filler
        pro = [proj_closures(sb, pro=True) for sb in range(4)]
        for cls in pro:
            cls[0]()          # loads
        for cls in pro:
            for cl in cls[1:]:
                cl()
        filler = []
        for sb in range(4, NSB):
            for cl in proj_closures(sb):
                filler.append((sb, cl))
        fpos = [0]

        def pump(n=1):
            hi = min(fpos[0] + n, len(filler))
            for i in range(fpos[0], hi):
                filler[i][1]()
            fpos[0] = hi

        def flush_blocks(upto):
            while fpos[0] < len(filler) and filler[fpos[0]][0] <= upto:
                filler[fpos[0]][1]()
                fpos[0] += 1

        # ------------------- attention + output projection ----------------
        def attn_pair(qg, hp, st8s, aqb):
            """Scores + exp + PV for heads (2hp, 2hp+1): scores run
            concurrently in row groups 0-1/2-3; PV accumulates into a
            2-bank pair tile, lagging scores by 2 key-blocks."""
            q0 = qg * 512
            pvh = pvp.tile([DK + 1, 2, 512], F32, name="pvh", tag="pv")
            emitters = []

            def drive(last=False):
                n = len(emitters)
                hi = n if last else max(n - 4, 0)
                for i in range(drive.done, hi):
                    emitters[i](i < 2, last and i >= n - 2)
                drive.done = hi
            drive.done = 0

            nkb = 4 * qg + 4
            for kb in range(nkb):        # one unit per 128-key block
                off = kb - 4 * qg
                c0 = 128 * max(off, 0)
                wd = 512 - c0
                sc = scp.tile([128, 2, 512], F32, name="sc", tag="sc")
                for hh in range(2):
                    nc.tensor.matmul(
                        sc[:, hh, c0:512],
                        kT4[64 * hh:64 * hh + DK, hp,
                            kb * 128:(kb + 1) * 128],
                        qT4[64 * hh:64 * hh + DK, hp, q0 + c0:q0 + 512],
                        start=True, stop=True)
                # one pair-wide exp (both heads, 2 PSUM banks) halves the
                # per-ACTIVATE fixed overhead
                ex = expp.tile([128, 2, 512], BF16, name="ex", tag="ex",
                               bufs=4)
                nc.scalar.activation(ex[:, :, 0:wd], sc[:, :, c0:512],
                                     EXPF, scale=SCL, bias=bsh[:, 0:1])
                if off >= 0:
                    # causal mask for the 128-wide diagonal square on
                    # the (idle) gpsimd: zero the q < k half post-exp
                    for hh in range(2):
                        nc.gpsimd.affine_select(
                            out=ex[:, hh, 0:128], in_=ex[:, hh, 0:128],
                            compare_op=mybir.AluOpType.is_ge, fill=0.0,
                            base=0, pattern=[[1, 128]],
                            channel_multiplier=-1)
                for hh in range(2):
                    emitters.append(
                        lambda stf, spf, kb=kb, hh=hh, c0=c0, wd=wd,
                        ex=ex:
                        nc.tensor.matmul(
                            pvh[:, hh, c0:512],
                            vtp[kb // 2][:, kb % 2, 2 * hp + hh, :],
                            ex[:, hh, 0:wd], start=stf, stop=spf))
                    drive()
                pump(1)
            drive(last=True)

            # evict unnormalized attn^T rows; stage the denominator rows
            st8s[hp] = p2t.tile([2, 512], F32, name="st8", tag="st8",
                                bufs=8)
            for hh in range(2):
                nc.vector.tensor_copy(aqb[64 * hh:64 * hh + 64, hp, :],
                                      pvh[0:DK, hh, :])
                dsb = p2t.tile([1, 512], F32, name="dsb", tag="dsb",
                               bufs=3)
                nc.vector.tensor_copy(dsb[:], pvh[DK:DK + 1, hh, :])
                nc.sync.dma_start(st8s[hp][hh:hh + 1, :], dsb[:])

        def make_deferred(qg, st8s, aqb):
            """Normalize + output projection closures for q-group qg;
            woven into the next q-group's pair loop (and, for the last
            q-group, its own pair loop)."""

            def norm_pair(hp):
                # per-pair fast reciprocal (~51 ULP, plenty for softmax
                # denominators) so norms need no whole-group barrier
                rc2 = p2t.tile([2, 512], F32, name="rc2", tag="rc2",
                               bufs=2)
                nc.vector.reciprocal_approx_fast(rc2[:], st8s[hp][:])
                for hh in range(2):
                    r0 = 64 * hh
                    rsb = p2t.tile([1, 512], F32, name="rsb", tag="rsb",
                                   bufs=2)
                    nc.sync.dma_start(rsb[:], rc2[hh:hh + 1, :])
                    rbc = p2t.tile([128, 512], F32, name="rbc",
                                   tag="rbc", bufs=2)
                    nc.gpsimd.partition_broadcast(rbc[:], rsb[:],
                                                  channels=128)
                    nc.vector.tensor_mul(aqb[r0:r0 + 64, hp, :],
                                         aqb[r0:r0 + 64, hp, :],
                                         rbc[r0:r0 + 64, :])

            def oproj(sbl):
                s0 = qg * 512 + sbl * 128
                yt = p2t.tile([128, 1024], F32, name="yt", tag="yt",
                              bufs=2)
                py = scp.tile([128, 2, 512], F32, name="py", tag="sc")
                for ih in range(2):
                    for j in range(4):
                        nc.tensor.matmul(
                            py[:, ih, :],
                            aqb[:, j, sbl * 128:(sbl + 1) * 128],
                            wo_s[:, j, ih * 512:(ih + 1) * 512],
                            start=(j == 0), stop=(j == 3))
                nc.vector.tensor_copy(yt[:], py[:])
                nc.sync.dma_start(yp[s0:s0 + 128, :], yt[:])

            norms = [lambda hp=hp: norm_pair(hp) for hp in range(4)]
            oprojs = [lambda sbl=sbl: oproj(sbl) for sbl in range(4)]
            return norms, oprojs

        prev_steps = []
        for qg in range(NQG):
            flush_blocks(4 * qg + 3)
            st8s = {}
            aqb = aqp.tile([128, 4, 512], BF16, name="aqb", tag="aqb",
                           bufs=2)
            norms, oprojs = make_deferred(qg, st8s, aqb)
            last = qg == NQG - 1
            for hp in range(4):
                attn_pair(qg, hp, st8s, aqb)
                # weave the previous q-group's normalize/oproj steps in
                # order (all norms strictly before any oproj)
                for stp in prev_steps[2 * hp:2 * hp + 2]:
                    stp()
                if last:
                    norms[hp]()           # own norm as soon as staged
            prev_steps = norms + oprojs
        for stp in prev_steps[4:]:        # last q-group's oprojs
            stp()

    nc.compile()
    return nc


def _prep_inputs(x, token_positions, wq, wk, wv, wo):
    bf16 = ml_dtypes.bfloat16
    # per-head permutation: [0,2,...,62, 1,3,...,63] (evens then odds)
    pi = np.concatenate([np.arange(0, DK, 2), np.arange(1, DK, 2)])
    perm = (np.arange(NH)[:, None] * DK + pi[None, :]).reshape(-1)
    wq_p = wq[perm, :]
    wk_p = wk[perm, :]

    pos = np.asarray(token_positions).astype(np.float32)
    thetas = (1.0 / (THETA ** (2.0 * np.arange(DK // 2, dtype=np.float32)
                               / DK))).astype(np.float32)
    ang = np.outer(pos, thetas).astype(np.float32)          # [S, 32]
    cos = np.tile(np.cos(ang), (1, HPG)).astype(np.float32)  # [S, 256]
    sin = np.tile(np.sin(ang), (1, HPG)).astype(np.float32)

    in_maps = []
    for core in range(8):
        b, g = core // 2, core % 2
        gs = slice(g * W, (g + 1) * W)
        in_maps.append({
            "xt": np.ascontiguousarray(x[b].T).astype(bf16),
            "wqt": np.ascontiguousarray(wq_p[gs, :].T).astype(bf16),
            "wkt": np.ascontiguousarray(wk_p[gs, :].T).astype(bf16),
            "wvt": np.ascontiguousarray(wv[gs, :].T).astype(bf16),
            "wot": np.ascontiguousarray(wo[:, gs].T).astype(bf16),
            "cosb": cos,
            "sinb": sin,
        })
    return in_maps


last_exec_time_ns = None


def _install_ntff_hook_shim():
    """This image's antenv lacks axon_hooks; wire the ctypes NTFF hook from
    trn_agent_boot so trace=True yields HW exec times."""
    import sys as _sys
    import types as _types
    try:
        from antenv import axon_hooks  # noqa: F401
        return
    except ImportError:
        pass
    from trn_agent_boot.trn_boot import _ntff_profile_via_ctypes
    hook = _ntff_profile_via_ctypes("/opt/axon/libaxon_pjrt.so")
    mod = _types.ModuleType("antenv.axon_hooks")
    mod.get_axon_ntff_profile_hook = lambda: hook
    _sys.modules["antenv.axon_hooks"] = mod


def kernel(x, token_positions, wq, wk, wv, wo):
    global last_exec_time_ns
    x = np.asarray(x, dtype=np.float32)
    token_positions = np.asarray(token_positions)
    wq = np.asarray(wq, dtype=np.float32)
    wk = np.asarray(wk, dtype=np.float32)
    wv = np.asarray(wv, dtype=np.float32)
    wo = np.asarray(wo, dtype=np.float32)

    if "nc" not in _cache:
        _cache["nc"] = build_nc()
    nc = _cache["nc"]

    in_maps = _prep_inputs(x, token_positions, wq, wk, wv, wo)
    res = None
    if TRACE:
        try:
            _install_ntff_hook_shim()
            res = run_bass_kernel_spmd(nc, in_maps, list(range(8)),
                                       trace=True,
                                       trace_cores=list(range(8)))
        except Exception as e:  # profiling must never sink correctness
            print(f"trace run failed ({type(e).__name__}: {e}); "
                  f"retrying untraced")
            res = None
    if res is None:
        res = run_bass_kernel_spmd(nc, in_maps, list(range(8)))
    last_exec_time_ns = res.exec_time_ns

    out = np.empty((B, S, D), dtype=np.float32)
    for b in range(B):
        out[b] = res.results[2 * b]["yp"] + res.results[2 * b + 1]["yp"]
    return out


# revision 18
# speedup vs baseline: 1.5088x; 1.5088x over previous
"""Causal multi-head self-attention with RoPE on 8 Trainium2 NeuronCores.

Sharding: core = (batch b, head-group g) with b = core//2, g = core%2.
Each core computes QKV projections for its batch element restricted to its
8 heads (512 of 1024 projection rows), RoPE, causal attention, and the
partial output projection y_g = attn_g @ wo[:, g*512:(g+1)*512].T.  The host
sums the two head-group partials per batch element.

v3 redesign (vs the 466us v2 baseline):
v2's trace showed the PE at half clock (HAM K=4/8) for ~270us: phase 2's
per-unit PE work (~0.5us) sat under the ACT exp rate (~0.7us/unit), the
micro-idles kept re-throttling the PE, and cold matmuls (427ns vs 213ns)
then became the critical path.  v3 merges the phases: the projection
pipeline for s-blocks 4..15 is decomposed into small closures that are
pumped between attention score/PV matmuls, so the PE always has dense
independent work while ACT exps drain.  Engine totals land ~balanced
(PE ~215us, ACT ~210us, DVE ~200us).
- scores for the head pair (2hp, 2hp+1) sit in row groups 0-1 / 2-3
  (contraction 64 at base partitions 0/64) and execute concurrently.
- PV accumulates both heads of a pair into one 2-bank PSUM tile with
  per-bank start/stop flags; pv matmuls lag scores by 2 key-blocks.
- PSUM: proj 2 banks, scores 2, {transpose-chain, oproj} shared 2, PV 2.
"""
import math
import os
from contextlib import ExitStack

import numpy as np
import ml_dtypes

import concourse.bass as bass
import concourse.tile as tile
from concourse import bacc, mybir
from concourse import masks
from concourse.bass_utils import run_bass_kernel_spmd

F32 = mybir.dt.float32
BF16 = mybir.dt.bfloat16
EXPF = mybir.ActivationFunctionType.Exp

D = 1024          # d_model
NH = 16           # heads total
DK = 64           # head dim
S = 2048          # sequence
B = 4             # batch
THETA = 10000.0
HPG = 8           # heads per group (2 groups over 8 cores with 4 batches)
W = HPG * DK      # 512: local projection width
NSB = S // 128    # 16 s-blocks
NQG = 4           # 512-wide q groups
ESH = -3.0        # exp shift: exp(s/8 - 3); cancels in normalization
SCL = 1.0 / math.sqrt(DK)

MODE = "v4-pairexp (bf16)"
TRACE = bool(int(os.environ.get("KTRACE", "0")))

_cache = {}


def build_nc():
    nc = bacc.Bacc(None, target_bir_lowering=False, debug=False)

    xt = nc.dram_tensor("xt", [D, S], BF16, kind="ExternalInput")
    wqt = nc.dram_tensor("wqt", [D, W], BF16, kind="ExternalInput")
    wkt = nc.dram_tensor("wkt", [D, W], BF16, kind="ExternalInput")
    wvt = nc.dram_tensor("wvt", [D, W], BF16, kind="ExternalInput")
    wot = nc.dram_tensor("wot", [W, D], BF16, kind="ExternalInput")
    cosb = nc.dram_tensor("cosb", [S, W // 2], F32, kind="ExternalInput")
    sinb = nc.dram_tensor("sinb", [S, W // 2], F32, kind="ExternalInput")
    yp = nc.dram_tensor("yp", [S, D], F32, kind="ExternalOutput")

    xt3 = xt[:].rearrange("(jo p) s -> p jo s", p=128)       # [128, 8, S]
    wqt3 = wqt[:].rearrange("(jo p) i -> p jo i", p=128)     # [128, 8, W]
    wkt3 = wkt[:].rearrange("(jo p) i -> p jo i", p=128)
    wvt3 = wvt[:].rearrange("(jo p) i -> p jo i", p=128)
    wot3 = wot[:].rearrange("(jo p) i -> p jo i", p=128)     # [128, 4, D]

    with tile.TileContext(nc, pool_alloc_mode="queue") as tc, \
            ExitStack() as ctx:
        persist = ctx.enter_context(tc.tile_pool(name="persist", bufs=1))
        identb = persist.tile([128, 128], BF16, name="identb")
        masks.make_identity(nc, identb)
        bsh = persist.tile([128, 1], F32, name="bsh")
        nc.gpsimd.memset(bsh, ESH)

        # persistent activations: q^T / k^T as [128, slab, S] (slab j holds
        # the 128 dims of heads (2j, 2j+1)); v s-major in s-block PAIRS with
        # a ones column per head for the softmax denominator.
        qT4 = persist.tile([128, 4, S], BF16, name="qT4")
        kT4 = persist.tile([128, 4, S], BF16, name="kT4")
        vtp = [persist.tile([128, 2, HPG, DK + 1], BF16, name=f"vtp{i}")
               for i in range(NSB // 2)]

        wp = ctx.enter_context(tc.tile_pool(name="wp", bufs=1))
        p1t = ctx.enter_context(tc.tile_pool(name="p1t", bufs=3))
        p2t = ctx.enter_context(tc.tile_pool(name="p2t", bufs=3))
        aqp = ctx.enter_context(tc.tile_pool(name="aqp", bufs=2))
        expp = ctx.enter_context(tc.tile_pool(name="exp", bufs=6))
        # PSUM: proj 2 banks; {score-pairs, transpose-chain, oproj-pair}
        # share one 2-slot pool of 2-bank slots; PV pair 2 banks.
        ppp = ctx.enter_context(tc.tile_pool(name="ppp", bufs=2,
                                             space="PSUM"))
        scp = ctx.enter_context(tc.tile_pool(name="scp", bufs=2,
                                             space="PSUM"))
        pvp = ctx.enter_context(tc.tile_pool(name="pvp", bufs=1,
                                             space="PSUM"))

        # ---- weights: wq as 8 per-chunk tiles on the scalar queue so the
        # first projection matmul waits on 128KB, not 1MB; k/v/o on gpsimd
        wq_j = [wp.tile([128, W], BF16, name=f"wq_j{jo}") for jo in range(8)]
        for jo in range(8):
            nc.scalar.dma_start(wq_j[jo][:], wqt3[:, jo, :])
        wk_s = wp.tile([128, 8, W], BF16, name="wk_s")
        wv_s = wp.tile([128, 8, W], BF16, name="wv_s")
        wo_s = wp.tile([128, 4, D], BF16, name="wo_s")
        nc.gpsimd.dma_start(wk_s[:], wkt3[:])
        nc.gpsimd.dma_start(wv_s[:], wvt3[:])
        nc.gpsimd.dma_start(wo_s[:], wot3[:])

        def rope(ps, outt, c3, s3):
            # ps: [128, W] PSUM (pre-RoPE proj, s-major, heads as
            # [evens(32) | odds(32)] blocks); outt: [128, W] SBUF bf16
            pe = ps.rearrange("p (h eo c) -> p h eo c", eo=2, c=32)
            ein, oin = pe[:, :, 0, :], pe[:, :, 1, :]
            oe = outt.rearrange("p (h eo c) -> p h eo c", eo=2, c=32)
            eout, oout = oe[:, :, 0, :], oe[:, :, 1, :]
            ra = p1t.tile([128, 8, 32], F32, name="ra", tag="ra")
            rb = p1t.tile([128, 8, 32], F32, name="rb", tag="rb")
            nc.vector.tensor_mul(ra, ein, c3)
            nc.vector.tensor_mul(rb, oin, s3)
            nc.vector.tensor_sub(eout, ra, rb)
            rc = p1t.tile([128, 8, 32], F32, name="rc", tag="rc")
            rd = p1t.tile([128, 8, 32], F32, name="rd", tag="rd")
            nc.vector.tensor_mul(rc, ein, s3)
            nc.vector.tensor_mul(rd, oin, c3)
            nc.vector.tensor_add(oout, rc, rd)

        def proj_closures(sb, pro=False):
            """Projection pipeline for s-block sb, as a list of small
            closures (each ~0.5-1us of one engine) pumped between
            attention matmuls.  In the prologue (pro=True), pq/pk live in
            halves of an (otherwise idle) scp pair slot so the PE never
            waits for a RoPE drain."""
            s0 = sb * 128
            st = {}

            def get_psum(key):
                if key not in st:
                    if pro and key in ("pq", "pk"):
                        if "qkpair" not in st:
                            st["qkpair"] = scp.tile([128, 2, 512], F32,
                                                    name="qkp", tag="sc")
                        st[key] = st["qkpair"][:, 0 if key == "pq" else 1, :]
                    else:
                        st[key] = ppp.tile([128, W], F32, name=key,
                                           tag="pp")
                return st[key]

            def load():
                xs = p1t.tile([128, 8, 128], BF16, name="xs", tag="xs")
                nc.sync.dma_start(xs[:], xt3[:, :, s0:s0 + 128])
                cs = p1t.tile([128, W // 2], F32, name="cs", tag="cs")
                nc.sync.dma_start(cs[:], cosb[s0:s0 + 128, :])
                sn = p1t.tile([128, W // 2], F32, name="sn", tag="sn")
                nc.sync.dma_start(sn[:], sinb[s0:s0 + 128, :])
                st["xs"], st["cs"], st["sn"] = xs, cs, sn

            def mk_mm(key, wsrc, lo, hi):
                def mm():
                    dst, xs = get_psum(key), st["xs"]
                    for jo in range(lo, hi):
                        if isinstance(wsrc, list):
                            w = wsrc[jo][:]
                        else:
                            w = wsrc[:, jo, :]
                        nc.tensor.matmul(dst[:], xs[:, jo, :], w,
                                         start=(jo == 0), stop=(jo == 7))
                return mm

            def ropeq():
                q_ro = p1t.tile([128, W], BF16, name="q_ro", tag="qro",
                                bufs=2)
                c3 = st["cs"].rearrange("p (h c) -> p h c", c=32)
                s3 = st["sn"].rearrange("p (h c) -> p h c", c=32)
                rope(st["pq"], q_ro, c3, s3)
                st["q_ro"] = q_ro

            def ropek():
                k_ro = p1t.tile([128, W], BF16, name="k_ro", tag="kro",
                                bufs=2)
                c3 = st["cs"].rearrange("p (h c) -> p h c", c=32)
                s3 = st["sn"].rearrange("p (h c) -> p h c", c=32)
                rope(st["pk"], k_ro, c3, s3)
                st["k_ro"] = k_ro

            def vev():
                # v eviction (+ ones column per head) on DVE
                v3 = vtp[sb // 2][:, sb % 2, :, :]       # [128, 8, 65]
                nc.vector.tensor_copy(
                    v3[:, :, 0:DK],
                    st["pv"].rearrange("p (h c) -> p h c", c=DK))
                nc.gpsimd.memset(v3[:, :, DK:DK + 1].bitcast(mybir.dt.uint16),
                                 0x3F80)

            def trans():
                # 8 chained bf16 transposes into one PSUM bank; evicted by
                # the ACT engine (measured: DVE strided PSUM reads slow the
                # exp PSUM port; ACT has the slack)
                ptr8 = scp.tile([128, 8, 128], BF16, name="ptr8", tag="sc")
                q_ro, k_ro = st["q_ro"], st["k_ro"]
                for pr in range(4):
                    nc.tensor.matmul(
                        ptr8[:, pr, :], q_ro[:, pr * 128:(pr + 1) * 128],
                        identb[:], is_transpose=True,
                        start=(pr == 0), stop=False)
                for pr in range(4):
                    nc.tensor.matmul(
                        ptr8[:, 4 + pr, :], k_ro[:, pr * 128:(pr + 1) * 128],
                        identb[:], is_transpose=True,
                        start=False, stop=(pr == 3))
                nc.scalar.copy(qT4[:, :, s0:s0 + 128], ptr8[:, 0:4, :])
                nc.scalar.copy(kT4[:, :, s0:s0 + 128], ptr8[:, 4:8, :])

            return [load,
                    mk_mm("pq", wq_j, 0, 4), mk_mm("pq", wq_j, 4, 8),
                    ropeq,
                    mk_mm("pk", wk_s, 0, 4), mk_mm("pk", wk_s, 4, 8),
                    ropek,
                    mk_mm("pv", wv_s, 0, 4), mk_mm("pv", wv_s, 4, 8),
                    vev, trans]

        # absorb the ~2.7us exp table load off the critical path
        wdum = p2t.tile([1, 1], F32, name="wdum", tag="wdum")
        nc.scalar.activation(wdum[:], bsh[0:1, 0:1], EXPF)

        # prologue: s-blocks 0-3 run eagerly (loads first so the sync
        # queue streams x while the PE chews); 4-15 queued as filler
        pro = [proj_closures(sb, pro=True) for sb in range(4)]
        for cls in pro:
            cls[0]()          # loads
        for cls in pro:
            for cl in cls[1:]:
                cl()
        filler = []
        for sb in range(4, NSB):
            for cl in proj_closures(sb):
                filler.append((sb, cl))
        fpos = [0]

        def pump(n=1):
            hi = min(fpos[0] + n, len(filler))
            for i in range(fpos[0], hi):
                filler[i][1]()
            fpos[0] = hi

        def flush_blocks(upto):
            while fpos[0] < len(filler) and filler[fpos[0]][0] <= upto:
                filler[fpos[0]][1]()
                fpos[0] += 1

        # ------------------- attention + output projection ----------------
        def attn_pair(qg, hp, st8s, aqb):
            """Scores + exp + PV for heads (2hp, 2hp+1): scores run
            concurrently in row groups 0-1/2-3; PV accumulates into a
            2-bank pair tile, lagging scores by 2 key-blocks."""
            q0 = qg * 512
            pvh = pvp.tile([DK + 1, 2, 512], F32, name="pvh", tag="pv")
            emitters = []

            def drive(last=False):
                n = len(emitters)
                hi = n if last else max(n - 4, 0)
                for i in range(drive.done, hi):
                    emitters[i](i < 2, last and i >= n - 2)
                drive.done = hi
            drive.done = 0

            nkb = 4 * qg + 4
            for kb in range(nkb):        # one unit per 128-key block
                off = kb - 4 * qg
                c0 = 128 * max(off, 0)
                wd = 512 - c0
                sc = scp.tile([128, 2, 512], F32, name="sc", tag="sc")
                for hh in range(2):
                    nc.tensor.matmul(
                        sc[:, hh, c0:512],
                        kT4[64 * hh:64 * hh + DK, hp,
                            kb * 128:(kb + 1) * 128],
                        qT4[64 * hh:64 * hh + DK, hp, q0 + c0:q0 + 512],
                        start=True, stop=True)
                # one pair-wide exp (both heads, 2 PSUM banks) halves the
                # per-ACTIVATE fixed overhead
                ex = expp.tile([128, 2, 512], BF16, name="ex", tag="ex",
                               bufs=4)
                nc.scalar.activation(ex[:, :, 0:wd], sc[:, :, c0:512],
                                     EXPF, scale=SCL, bias=bsh[:, 0:1])
                if off >= 0:
                    # causal mask for the 128-wide diagonal square on
                    # the (idle) gpsimd: zero the q < k half post-exp
                    for hh in range(2):
                        nc.gpsimd.affine_select(
                            out=ex[:, hh, 0:128], in_=ex[:, hh, 0:128],
                            compare_op=mybir.AluOpType.is_ge, fill=0.0,
                            base=0, pattern=[[1, 128]],
                            channel_multiplier=-1)
                for hh in range(2):
                    emitters.append(
                        lambda stf, spf, kb=kb, hh=hh, c0=c0, wd=wd,
                        ex=ex:
                        nc.tensor.matmul(
                            pvh[:, hh, c0:512],
                            vtp[kb // 2][:, kb % 2, 2 * hp + hh, :],
                            ex[:, hh, 0:wd], start=stf, stop=spf))
                    drive()
                pump(1)
            drive(last=True)

            # evict unnormalized attn^T rows; stage the denominator rows
            st8s[hp] = p2t.tile([2, 512], F32, name="st8", tag="st8",
                                bufs=8)
            for hh in range(2):
                nc.vector.tensor_copy(aqb[64 * hh:64 * hh + 64, hp, :],
                                      pvh[0:DK, hh, :])
                dsb = p2t.tile([1, 512], F32, name="dsb", tag="dsb",
                               bufs=3)
                nc.vector.tensor_copy(dsb[:], pvh[DK:DK + 1, hh, :])
                nc.sync.dma_start(st8s[hp][hh:hh + 1, :], dsb[:])

        def make_deferred(qg, st8s, aqb):
            """Normalize + output projection closures for q-group qg;
            woven into the next q-group's pair loop (and, for the last
            q-group, its own pair loop)."""

            def norm_pair(hp):
                # per-pair fast reciprocal (~51 ULP, plenty for softmax
                # denominators) so norms need no whole-group barrier
                rc2 = p2t.tile([2, 512], F32, name="rc2", tag="rc2",
                               bufs=2)
                nc.vector.reciprocal_approx_fast(rc2[:], st8s[hp][:])
                for hh in range(2):
                    r0 = 64 * hh
                    rsb = p2t.tile([1, 512], F32, name="rsb", tag="rsb",
                                   bufs=2)
                    nc.sync.dma_start(rsb[:], rc2[hh:hh + 1, :])
                    rbc = p2t.tile([128, 512], F32, name="rbc",
                                   tag="rbc", bufs=2)
                    nc.gpsimd.partition_broadcast(rbc[:], rsb[:],
                                                  channels=128)
                    nc.vector.tensor_mul(aqb[r0:r0 + 64, hp, :],
                                         aqb[r0:r0 + 64, hp, :],
                                         rbc[r0:r0 + 64, :])

            def oproj(sbl):
                s0 = qg * 512 + sbl * 128
                yt = p2t.tile([128, 1024], F32, name="yt", tag="yt",
                              bufs=2)
                py = scp.tile([128, 2, 512], F32, name="py", tag="sc")
                for ih in range(2):
                    for j in range(4):
                        nc.tensor.matmul(
                            py[:, ih, :],
                            aqb[:, j, sbl * 128:(sbl + 1) * 128],
                            wo_s[:, j, ih * 512:(ih + 1) * 512],
                            start=(j == 0), stop=(j == 3))
                nc.vector.tensor_copy(yt[:], py[:])
                nc.sync.dma_start(yp[s0:s0 + 128, :], yt[:])

            norms = [lambda hp=hp: norm_pair(hp) for hp in range(4)]
            oprojs = [lambda sbl=sbl: oproj(sbl) for sbl in range(4)]
            return norms, oprojs

        prev_steps = []
        for qg in range(NQG):
            flush_blocks(4 * qg + 3)
            st8s = {}
            aqb = aqp.tile([128, 4, 512], BF16, name="aqb", tag="aqb",
                           bufs=2)
            norms, oprojs = make_deferred(qg, st8s, aqb)
            last = qg == NQG - 1
            for hp in range(4):
                attn_pair(qg, hp, st8s, aqb)
                # weave the previous q-group's normalize/oproj steps in
                # order (all norms strictly before any oproj)
                for stp in prev_steps[2 * hp:2 * hp + 2]:
                    stp()
                if last:
                    norms[hp]()           # own norm as soon as staged
            prev_steps = norms + oprojs
        for stp in prev_steps[4:]:        # last q-group's oprojs
            stp()

    nc.compile()
    return nc


def _prep_inputs(x, token_positions, wq, wk, wv, wo):
    bf16 = ml_dtypes.bfloat16
    # per-head permutation: [0,2,...,62, 1,3,...,63] (evens then odds)
    pi = np.concatenate([np.arange(0, DK, 2), np.arange(1, DK, 2)])
    perm = (np.arange(NH)[:, None] * DK + pi[None, :]).reshape(-1)
    wq_p = wq[perm, :]
    wk_p = wk[perm, :]

    pos = np.asarray(token_positions).astype(np.float32)
    thetas = (1.0 / (THETA ** (2.0 * np.arange(DK // 2, dtype=np.float32)
                               / DK))).astype(np.float32)
    ang = np.outer(pos, thetas).astype(np.float32)          # [S, 32]
    cos = np.tile(np.cos(ang), (1, HPG)).astype(np.float32)  # [S, 256]
    sin = np.tile(np.sin(ang), (1, HPG)).astype(np.float32)

    in_maps = []
    for core in range(8):
        b, g = core // 2, core % 2
        gs = slice(g * W, (g + 1) * W)
        in_maps.append({
            "xt": np.ascontiguousarray(x[b].T).astype(bf16),
            "wqt": np.ascontiguousarray(wq_p[gs, :].T).astype(bf16),
            "wkt": np.ascontiguousarray(wk_p[gs, :].T).astype(bf16),
            "wvt": np.ascontiguousarray(wv[gs, :].T).astype(bf16),
            "wot": np.ascontiguousarray(wo[:, gs].T).astype(bf16),
            "cosb": cos,
            "sinb": sin,
        })
    return in_maps


last_exec_time_ns = None


def _install_ntff_hook_shim():
    """This image's antenv lacks axon_hooks; wire the ctypes NTFF hook from
    trn_agent_boot so trace=True yields HW exec times."""
    import sys as _sys
    import types as _types
    try:
        from antenv import axon_hooks  # noqa: F401
        return
    except ImportError:
        pass
    from trn_agent_boot.trn_boot import _ntff_profile_via_ctypes
    hook = _ntff_profile_via_ctypes("/opt/axon/libaxon_pjrt.so")
    mod = _types.ModuleType("antenv.axon_hooks")
    mod.get_axon_ntff_profile_hook = lambda: hook
    _sys.modules["antenv.axon_hooks"] = mod


def kernel(x, token_positions, wq, wk, wv, wo):
    global last_exec_time_ns
    x = np.asarray(x, dtype=np.float32)
    token_positions = np.asarray(token_positions)
    wq = np.asarray(wq, dtype=np.float32)
    wk = np.asarray(wk, dtype=np.float32)
    wv = np.asarray(wv, dtype=np.float32)
    wo = np.asarray(wo, dtype=np.float32)

    if "nc" not in _cache:
        _cache["nc"] = build_nc()
    nc = _cache["nc"]

    in_maps = _prep_inputs(x, token_positions, wq, wk, wv, wo)
    res = None
    if TRACE:
        try:
            _install_ntff_hook_shim()
            res = run_bass_kernel_spmd(nc, in_maps, list(range(8)),
                                       trace=True,
                                       trace_cores=list(range(8)))
        except Exception as e:  # profiling must never sink correctness
            print(f"trace run failed ({type(e).__name__}: {e}); "
                  f"retrying untraced")
            res = None
    if res is None:
        res = run_bass_kernel_spmd(nc, in_maps, list(range(8)))
    last_exec_time_ns = res.exec_time_ns

    out = np.empty((B, S, D), dtype=np.float32)
    for b in range(B):
        out[b] = res.results[2 * b]["yp"] + res.results[2 * b + 1]["yp"]
    return out
